# revision 1
# baseline (speedup 1.0000x reference)
"""Trainium2 Bass kernel for nn_InputRotationWrapper: y = WHT(x) @ W^T + b.

Algebraic fold: WHT (normalized Walsh-Hadamard along feature dim, H symmetric)
commutes into the weight: y = (x H) W^T = x (W H)^T.  The device therefore runs
a pure GEMM  y = x @ Wr^T + b  with Wr = WHT(W) computed once on the host.

Distribution: data-parallel over the 8192 tokens across 8 NeuronCores (1024
tokens each); Wr is replicated.  Each core computes its output slice
transposed (yT[o, t], o on partitions) so every DMA is fully contiguous:
  - x^T shard  [4096 d, 1024 t]  resident in SBUF as fp32r (16.7 MB)
  - Wr packed  [32 ob, 128 d_in, 32 d_chunk, 128 o]  streamed per o-block
  - out yT     [4096 o, 1024 t]  written per o-block
Matmul dtype is float32r (full-rate on the PE at N>=256, ~1.5e-4 rel err).
Bias is fused into the PSUM->SBUF eviction via ScalarE activation.
"""
import sys

for _p in ("/opt/trn_rl_repo", "/root/.axon_site/_ro/trn_rl_repo"):
    if _p not in sys.path:
        sys.path.insert(0, _p)

import numpy as np

D = 4096          # feature dim (= rotation size)
TOKENS = 8192     # 4 * 2048
N_CORES = 8
T_CORE = TOKENS // N_CORES   # 1024 tokens per core
P = 128           # partitions
DC = D // P       # 32 contraction chunks
OB = D // P       # 32 output blocks
T_HALF = 512      # moving free-dim per matmul (fp32 max)

_compiled = None  # (nc, tmpdir) cache


def _matmul_hadU_np(x: np.ndarray) -> np.ndarray:
    """Normalized WHT along the last axis — exact port of the reference
    recursive-butterfly (K == 1 branch), in float64."""
    n = x.shape[-1]
    shape = x.shape
    v = x.reshape(-1, n, 1)
    while v.shape[1] > 1:
        b_, m, c = v.shape
        v = v.reshape(b_, m // 2, 2, c)
        a, b = v[:, :, 0, :], v[:, :, 1, :]
        v = np.concatenate([a + b, a - b], axis=-1)
    return v.reshape(shape) / np.sqrt(n)


def _build_nc():
    import concourse.tile as tile
    from concourse import bacc, mybir

    dt = mybir.dt
    nc = bacc.Bacc(None, target_bir_lowering=False)

    xt_d = nc.dram_tensor("xt", [D, T_CORE], dt.float32, kind="ExternalInput")
    w_d = nc.dram_tensor("w", [OB, P, DC, P], dt.float32, kind="ExternalInput")
    b_d = nc.dram_tensor("bias", [P, OB], dt.float32, kind="ExternalInput")
    y_d = nc.dram_tensor("yt", [D, T_CORE], dt.float32, kind="ExternalOutput")

    G0 = 4   # o-blocks processed c-outer in the startup group: 8 matmuls
             # become ready per arriving x tile, saturating the PE while the
             # 16.7 MB x shard streams in.  Uses all 8 PSUM banks.
    QC = 8   # W streamed as quarter-tiles of 8 contraction chunks (512 KB):
             # fine arrival granularity at startup, and quarter slots free
             # mid-group so the next block's W prefetches without a stall.
    NQ = DC // QC

    with tile.TileContext(nc) as tc:
        with (
            tc.tile_pool(name="xp", bufs=1) as xp,
            tc.tile_pool(name="wp", bufs=17, space="SBUF") as wp,
            tc.tile_pool(name="bp", bufs=1) as bp,
            tc.tile_pool(name="op", bufs=2) as op,
            tc.tile_pool(name="pp", bufs=G0, space="PSUM") as pp,
        ):
            b_sb = bp.tile([P, OB], dt.float32)

            def load_wq(ob, q):
                wq = wp.tile([P, QC, P], dt.float32r, tag="w", name=f"w_{ob}_{q}")
                nc.sync.dma_start(
                    wq[:], w_d[ob, :, q * QC:(q + 1) * QC, :].bitcast(dt.float32r)
                )
                return wq

            def load_x(c):
                t = xp.tile([P, T_CORE], dt.float32r, tag=f"x{c}", name=f"x_{c}")
                src = xt_d[c * P:(c + 1) * P, :].bitcast(dt.float32r)
                nc.sync.dma_start(t[:, 0:T_HALF], src[:, 0:T_HALF])
                nc.sync.dma_start(t[:, T_HALF:T_CORE], src[:, T_HALF:T_CORE])
                return t

            def mms(ps, wq, c, x_t):
                lhsT = wq[:, c % QC, :]
                nc.tensor.matmul(
                    ps[:, 0:T_HALF], lhsT, x_t[:, 0:T_HALF],
                    start=(c == 0), stop=(c == DC - 1),
                )
                nc.tensor.matmul(
                    ps[:, T_HALF:T_CORE], lhsT, x_t[:, T_HALF:T_CORE],
                    start=(c == 0), stop=(c == DC - 1),
                )

            def evict(ob, ps):
                o_sb = op.tile([P, T_CORE], dt.float32, tag="o", name=f"o_{ob}")
                nc.scalar.activation(
                    o_sb[:], ps[:],
                    mybir.ActivationFunctionType.Identity,
                    bias=b_sb[:, ob:ob + 1],
                )
                nc.sync.dma_start(y_d[ob * P:(ob + 1) * P, :], o_sb[:])

            # DMA issue order approximates completion order: the c=0 pieces
            # first (w*q0 + x0), then one W quarter per x tile so each
            # quarter batch q lands well before its first consumer (c = 8q).
            x_tiles = [None] * DC
            w0 = [[None] * NQ for _ in range(G0)]
            w0[0][0] = load_wq(0, 0)
            x_tiles[0] = load_x(0)
            for ob in range(1, G0):
                w0[ob][0] = load_wq(ob, 0)
            nc.sync.dma_start(b_sb[:], b_d[:])
            pend = [(ob, q) for q in range(1, NQ) for ob in range(G0)]
            for c in range(1, DC):
                x_tiles[c] = load_x(c)
                if pend:
                    ob, q = pend.pop(0)
                    w0[ob][q] = load_wq(ob, q)

            ps0 = [
                pp.tile([P, T_CORE], dt.float32, tag="ps", name=f"ps0_{i}")
                for i in range(G0)
            ]
            for c in range(DC):
                for ob in range(G0):
                    mms(ps0[ob], w0[ob][c // QC], c, x_tiles[c])
            for ob in range(G0):
                evict(ob, ps0[ob])

            # steady state: one o-block at a time, W quarters prefetched
            for ob in range(G0, OB):
                wqs = [load_wq(ob, q) for q in range(NQ)]
                ps = pp.tile([P, T_CORE], dt.float32, tag="ps", name=f"ps_{ob}")
                for c in range(DC):
                    mms(ps, wqs[c // QC], c, x_tiles[c])
                evict(ob, ps)

    nc.compile()
    return nc


def _get_nc():
    global _compiled
    if _compiled is None:
        _compiled = _build_nc()
    return _compiled


def _prep_inputs(x, W, b):
    x = np.asarray(x, dtype=np.float32)
    W = np.asarray(W, dtype=np.float32)
    b = np.asarray(b, dtype=np.float32)

    Wr = _matmul_hadU_np(W.astype(np.float64)).astype(np.float32)  # [o, d]
    # W_pack[ob, p, c, j] = Wr[ob*128 + j, c*128 + p]
    w_pack = np.ascontiguousarray(
        Wr.reshape(OB, P, DC, P).transpose(0, 3, 2, 1)
    )
    b_pack = np.ascontiguousarray(b.reshape(OB, P).T)  # [128, 32]

    xt = np.ascontiguousarray(
        x.reshape(N_CORES, T_CORE, D).transpose(0, 2, 1)
    )  # [8, 4096, 1024]

    in_maps = [
        {"xt": xt[c], "w": w_pack, "bias": b_pack} for c in range(N_CORES)
    ]
    return in_maps


def _assemble(results):
    # yt per core: [4096 o, 1024 t] -> y[t, o]
    parts = [np.ascontiguousarray(r["yt"].T) for r in results]
    y = np.concatenate(parts, axis=0)  # [8192, 4096]
    return y.reshape(4, 2048, D)


def _run(x, W, b, **spmd_kwargs):
    from concourse.bass_utils import run_bass_kernel_spmd

    nc = _get_nc()
    in_maps = _prep_inputs(x, W, b)
    res = run_bass_kernel_spmd(nc, in_maps, list(range(N_CORES)), **spmd_kwargs)
    return _assemble(res.results), res


def kernel(x, W, b):
    out, _ = _run(x, W, b)
    return out



# revision 2
# speedup vs baseline: 1.2526x; 1.2526x over previous
"""Trainium2 Bass kernel for nn_InputRotationWrapper: y = WHT(x) @ W^T + b.

Algebraic fold: WHT (normalized Walsh-Hadamard along feature dim, H symmetric)
commutes into the weight: y = (x H) W^T = x (W H)^T.  The device therefore runs
a pure GEMM  y = x @ Wr^T + b  with Wr = WHT(W) computed once on the host.

Distribution: data-parallel over the 8192 tokens across 8 NeuronCores (1024
tokens each); Wr is replicated.  Each core computes its output slice
transposed (yT[o, t], o on partitions) so every DMA is fully contiguous:
  - x^T shard  [4096 d, 1024 t]  resident in SBUF as fp16 (8.4 MB)
  - Wr packed  [32 ob, 128 d_in, 32 d_chunk, 128 o]  fp16, streamed per o-block
  - out yT     [4096 o, 1024 t]  fp32, written per o-block

Matmul dtype is float16: full PE rate (1 row/cycle), and the per-matmul
LDWEIGHTS (128x128 stationary tile) takes ~112ns at 2B/row vs fp32r's 224ns,
so it hides completely behind the previous 512-row matmul (213ns) in the PE
shadow weight buffer.  fp32r's 224+44ns load path gated the baseline at a
272ns cadence; fp16 restores the 213ns roofline cadence.
Bias is fused into the PSUM->SBUF eviction via ScalarE activation.
"""
import sys

for _p in ("/opt/trn_rl_repo", "/root/.axon_site/_ro/trn_rl_repo"):
    if _p not in sys.path:
        sys.path.insert(0, _p)

import numpy as np

D = 4096          # feature dim (= rotation size)
TOKENS = 8192     # 4 * 2048
N_CORES = 8
T_CORE = TOKENS // N_CORES   # 1024 tokens per core
P = 128           # partitions
DC = D // P       # 32 contraction chunks
OB = D // P       # 32 output blocks
T_HALF = 512      # moving free-dim per matmul (hw max)

_compiled = None  # (nc, tmpdir) cache


def _matmul_hadU_np(x: np.ndarray) -> np.ndarray:
    """Normalized WHT along the last axis — exact port of the reference
    recursive-butterfly (K == 1 branch), in float64."""
    n = x.shape[-1]
    shape = x.shape
    v = x.reshape(-1, n, 1)
    while v.shape[1] > 1:
        b_, m, c = v.shape
        v = v.reshape(b_, m // 2, 2, c)
        a, b = v[:, :, 0, :], v[:, :, 1, :]
        v = np.concatenate([a + b, a - b], axis=-1)
    return v.reshape(shape) / np.sqrt(n)


def _build_nc():
    import concourse.tile as tile
    from concourse import bacc, mybir

    dt = mybir.dt
    nc = bacc.Bacc(None, target_bir_lowering=False)

    xt_d = nc.dram_tensor("xt", [D, T_CORE], dt.float16, kind="ExternalInput")
    w_d = nc.dram_tensor("w", [OB, P, DC, P], dt.float16, kind="ExternalInput")
    b_d = nc.dram_tensor("bias", [P, OB], dt.float32, kind="ExternalInput")
    y_d = nc.dram_tensor("yt", [D, T_CORE], dt.float32, kind="ExternalOutput")

    G0 = 4   # o-blocks processed c-outer in the startup group: 8 matmuls
             # become ready per arriving x tile, saturating the PE while the
             # 8.4 MB x shard streams in.  Uses all 8 PSUM banks.
    QC = 8   # W streamed as quarter-tiles of 8 contraction chunks (256 KB):
             # fine arrival granularity at startup, and quarter slots free
             # mid-group so the next block's W prefetches without a stall.
    NQ = DC // QC

    with tile.TileContext(nc) as tc:
        with (
            tc.tile_pool(name="xp", bufs=1) as xp,
            tc.tile_pool(name="wp", bufs=17, space="SBUF") as wp,
            tc.tile_pool(name="bp", bufs=1) as bp,
            tc.tile_pool(name="op", bufs=4) as op,
            tc.tile_pool(name="pp", bufs=G0, space="PSUM") as pp,
        ):
            b_sb = bp.tile([P, OB], dt.float32)

            def load_wq(ob, q):
                wq = wp.tile([P, QC, P], dt.float16, tag="w", name=f"w_{ob}_{q}")
                nc.sync.dma_start(
                    wq[:], w_d[ob, :, q * QC:(q + 1) * QC, :]
                )
                return wq

            def load_x(c):
                t = xp.tile([P, T_CORE], dt.float16, tag=f"x{c}", name=f"x_{c}")
                src = xt_d[c * P:(c + 1) * P, :]
                nc.sync.dma_start(t[:, 0:T_HALF], src[:, 0:T_HALF])
                nc.sync.dma_start(t[:, T_HALF:T_CORE], src[:, T_HALF:T_CORE])
                return t

            def mms(ps, wq, c, x_t):
                lhsT = wq[:, c % QC, :]
                nc.tensor.matmul(
                    ps[:, 0:T_HALF], lhsT, x_t[:, 0:T_HALF],
                    start=(c == 0), stop=(c == DC - 1),
                )
                nc.tensor.matmul(
                    ps[:, T_HALF:T_CORE], lhsT, x_t[:, T_HALF:T_CORE],
                    start=(c == 0), stop=(c == DC - 1),
                )

            def evict(ob, ps):
                # per-half eviction: half 0 drains while the PE is still
                # working elsewhere; shrinks the end-of-kernel tail.
                for h in range(2):
                    sl = slice(h * T_HALF, (h + 1) * T_HALF)
                    o_sb = op.tile([P, T_HALF], dt.float32, tag="o",
                                   name=f"o_{ob}_{h}")
                    nc.scalar.activation(
                        o_sb[:], ps[:, sl],
                        mybir.ActivationFunctionType.Identity,
                        bias=b_sb[:, ob:ob + 1],
                    )
                    nc.sync.dma_start(y_d[ob * P:(ob + 1) * P, sl], o_sb[:])

            # DMA issue order approximates completion order: the c=0 pieces
            # first (w*q0 + x0), then one W quarter per x tile so each
            # quarter batch q lands well before its first consumer (c = 8q).
            x_tiles = [None] * DC
            w0 = [[None] * NQ for _ in range(G0)]
            w0[0][0] = load_wq(0, 0)
            x_tiles[0] = load_x(0)
            for ob in range(1, G0):
                w0[ob][0] = load_wq(ob, 0)
            nc.sync.dma_start(b_sb[:], b_d[:])
            pend = [(ob, q) for q in range(1, NQ) for ob in range(G0)]
            for c in range(1, DC):
                x_tiles[c] = load_x(c)
                if pend:
                    ob, q = pend.pop(0)
                    w0[ob][q] = load_wq(ob, q)

            ps0 = [
                pp.tile([P, T_CORE], dt.float32, tag="ps", name=f"ps0_{i}")
                for i in range(G0)
            ]
            for c in range(DC):
                for ob in range(G0):
                    mms(ps0[ob], w0[ob][c // QC], c, x_tiles[c])
            for ob in range(G0):
                evict(ob, ps0[ob])

            # steady state: one o-block at a time, W quarters prefetched
            for ob in range(G0, OB):
                wqs = [load_wq(ob, q) for q in range(NQ)]
                ps = pp.tile([P, T_CORE], dt.float32, tag="ps", name=f"ps_{ob}")
                for c in range(DC):
                    mms(ps, wqs[c // QC], c, x_tiles[c])
                evict(ob, ps)

    nc.compile()
    return nc


def _get_nc():
    global _compiled
    if _compiled is None:
        _compiled = _build_nc()
    return _compiled


def _prep_inputs(x, W, b):
    x = np.asarray(x, dtype=np.float32)
    W = np.asarray(W, dtype=np.float32)
    b = np.asarray(b, dtype=np.float32)

    Wr = _matmul_hadU_np(W.astype(np.float64))  # [o, d] float64
    # W_pack[ob, p, c, j] = Wr[ob*128 + j, c*128 + p]
    w_pack = np.ascontiguousarray(
        Wr.reshape(OB, P, DC, P).transpose(0, 3, 2, 1).astype(np.float16)
    )
    b_pack = np.ascontiguousarray(b.reshape(OB, P).T)  # [128, 32]

    xt = np.ascontiguousarray(
        x.reshape(N_CORES, T_CORE, D).transpose(0, 2, 1).astype(np.float16)
    )  # [8, 4096, 1024]

    in_maps = [
        {"xt": xt[c], "w": w_pack, "bias": b_pack} for c in range(N_CORES)
    ]
    return in_maps


def _assemble(results):
    # yt per core: [4096 o, 1024 t] -> y[t, o]
    parts = [np.ascontiguousarray(r["yt"].T) for r in results]
    y = np.concatenate(parts, axis=0)  # [8192, 4096]
    return y.reshape(4, 2048, D)


def _run(x, W, b, **spmd_kwargs):
    from concourse.bass_utils import run_bass_kernel_spmd

    nc = _get_nc()
    in_maps = _prep_inputs(x, W, b)
    res = run_bass_kernel_spmd(nc, in_maps, list(range(N_CORES)), **spmd_kwargs)
    return _assemble(res.results), res


def kernel(x, W, b):
    out, _ = _run(x, W, b)
    return out


# revision 6
# speedup vs baseline: 1.2760x; 1.0186x over previous
"""Trainium2 Bass kernel for nn_InputRotationWrapper: y = WHT(x) @ W^T + b.

Algebraic fold: WHT (normalized Walsh-Hadamard along feature dim, H symmetric)
commutes into the weight: y = (x H) W^T = x (W H)^T.  The device therefore runs
a pure GEMM  y = x @ Wr^T + b  with Wr = WHT(W) computed once on the host.

Distribution: data-parallel over the 8192 tokens across 8 NeuronCores (1024
tokens each); Wr is replicated.  Each core computes its output slice
transposed (yT[o, t], o on partitions) so every DMA is fully contiguous:
  - x^T shard  [4096 d, 1024 t]  resident in SBUF as fp16 (8.4 MB)
  - Wr packed  [32 ob, 128 d_in, 32 d_chunk, 128 o]  fp16, streamed per o-block
  - out yT     [4096 o, 1024 t]  fp32, written per o-block

Matmul dtype is float16: full PE rate (1 row/cycle), and the per-matmul
LDWEIGHTS (128x128 stationary tile) takes ~100ns at 2B/row vs fp32r's 224ns,
so it hides completely in the PE shadow weight buffer behind the previous
512-row matmul (213ns).  fp32r's 224+44ns load path gated the baseline at a
272ns cadence; fp16 runs at the 213ns roofline cadence.

DMA triggers cost ~620ns each on a sequencer, so x is loaded via the Scalar
(Activation) HWDGE queue while W streams on the Sync queue — two parallel
trigger chains — and transfer sizes ramp up (1,1,2,4,4... c-chunks) so the
PE pipeline fills early without trigger-rate stalls.
Bias is fused into the PSUM->SBUF eviction via ScalarE activation.
"""
import sys

for _p in ("/opt/trn_rl_repo", "/root/.axon_site/_ro/trn_rl_repo"):
    if _p not in sys.path:
        sys.path.insert(0, _p)

import numpy as np

D = 4096          # feature dim (= rotation size)
TOKENS = 8192     # 4 * 2048
N_CORES = 8
T_CORE = TOKENS // N_CORES   # 1024 tokens per core
P = 128           # partitions
DC = D // P       # 32 contraction chunks
OB = D // P       # 32 output blocks
T_HALF = 512      # moving free-dim per matmul (hw max)

_compiled = None


def _matmul_hadU_np(x: np.ndarray) -> np.ndarray:
    """Normalized WHT along the last axis — exact port of the reference
    recursive-butterfly (K == 1 branch), in float64."""
    n = x.shape[-1]
    shape = x.shape
    v = x.reshape(-1, n, 1)
    while v.shape[1] > 1:
        b_, m, c = v.shape
        v = v.reshape(b_, m // 2, 2, c)
        a, b = v[:, :, 0, :], v[:, :, 1, :]
        v = np.concatenate([a + b, a - b], axis=-1)
    return v.reshape(shape) / np.sqrt(n)


def _build_nc():
    import concourse.tile as tile
    from concourse import bacc, mybir

    dt = mybir.dt
    nc = bacc.Bacc(None, target_bir_lowering=False)

    xt_d = nc.dram_tensor("xt", [P, DC, T_CORE], dt.float16, kind="ExternalInput")
    w_d = nc.dram_tensor("w", [OB, P, DC, P], dt.float16, kind="ExternalInput")
    b_d = nc.dram_tensor("bias", [P, OB], dt.float32, kind="ExternalInput")
    y_d = nc.dram_tensor("yt", [D, T_CORE], dt.float32, kind="ExternalOutput")

    G0 = 4   # o-blocks processed c-outer in the startup group: 8 matmuls
             # become ready per arriving x chunk, saturating the PE while the
             # 8.4 MB x shard streams in.  Uses all 8 PSUM banks.
    QC = 8   # startup W granularity: quarter-tiles of 8 contraction chunks
    HC = 16  # steady-state W granularity: half-tiles (fewer DMA triggers)
    NQ = DC // QC

    # x DMA schedule: (start_chunk, n_chunks) with sizes ramping up so the
    # first matmuls unblock ASAP but the whole stream needs few triggers.
    x_sched = [(0, 1), (1, 1), (2, 2), (4, 4), (8, 4), (12, 4), (16, 4),
               (20, 4), (24, 4), (28, 4)]

    with tile.TileContext(nc) as tc:
        with (
            tc.tile_pool(name="xp", bufs=1) as xp,
            tc.tile_pool(name="wp", bufs=18, space="SBUF") as wp,
            tc.tile_pool(name="bp", bufs=1) as bp,
            tc.tile_pool(name="op", bufs=4) as op,
            tc.tile_pool(name="pp", bufs=G0, space="PSUM") as pp,
        ):
            b_sb = bp.tile([P, OB], dt.float32)

            def load_wq(ob, q):
                wq = wp.tile([P, QC, P], dt.float16, tag="w", name=f"w_{ob}_{q}")
                nc.sync.dma_start(wq[:], w_d[ob, :, q * QC:(q + 1) * QC, :])
                return wq

            def load_wh(ob, h):
                wh = wp.tile([P, HC, P], dt.float16, tag="w2", name=f"wh_{ob}_{h}")
                nc.sync.dma_start(wh[:], w_d[ob, :, h * HC:(h + 1) * HC, :])
                return wh

            # x chunks: one SBUF tile per c-chunk, but DMAs grouped per
            # x_sched entry on the Scalar HWDGE queue (parallel to Sync).
            x_tiles = [None] * DC

            def load_x_group(c0, n):
                t = xp.tile([P, n, T_CORE], dt.float16, tag=f"x{c0}",
                            name=f"x_{c0}")
                nc.scalar.dma_start(t[:], xt_d[:, c0:c0 + n, :])
                for i in range(n):
                    x_tiles[c0 + i] = t[:, i, :]

            def mms(ps, lhsT, c, x_t):
                nc.tensor.matmul(
                    ps[:, 0:T_HALF], lhsT, x_t[:, 0:T_HALF],
                    start=(c == 0), stop=(c == DC - 1),
                )
                nc.tensor.matmul(
                    ps[:, T_HALF:T_CORE], lhsT, x_t[:, T_HALF:T_CORE],
                    start=(c == 0), stop=(c == DC - 1),
                )

            def evict(ob, ps):
                # per-half eviction: half 0 drains while the PE finishes
                # half 1; shrinks the end-of-kernel tail.
                for h in range(2):
                    sl = slice(h * T_HALF, (h + 1) * T_HALF)
                    o_sb = op.tile([P, T_HALF], dt.float32, tag="o",
                                   name=f"o_{ob}_{h}")
                    nc.scalar.activation(
                        o_sb[:], ps[:, sl],
                        mybir.ActivationFunctionType.Identity,
                        bias=b_sb[:, ob:ob + 1],
                    )
                    nc.sync.dma_start(y_d[ob * P:(ob + 1) * P, sl], o_sb[:])

            # issue order approximates arrival-need order.  Sync queue: W
            # quarters for the startup group, bias.  Scalar queue: x groups.
            w0 = [[None] * NQ for _ in range(G0)]
            w0[0][0] = load_wq(0, 0)
            load_x_group(*x_sched[0])
            for ob in range(1, G0):
                w0[ob][0] = load_wq(ob, 0)
            for sched in x_sched[1:]:
                load_x_group(*sched)
            nc.sync.dma_start(b_sb[:], b_d[:])
            for q in range(1, NQ):
                for ob in range(G0):
                    w0[ob][q] = load_wq(ob, q)

            ps0 = [
                pp.tile([P, T_CORE], dt.float32, tag="ps", name=f"ps0_{i}")
                for i in range(G0)
            ]
            for c in range(DC):
                for ob in range(G0):
                    mms(ps0[ob], w0[ob][c // QC][:, c % QC, :], c, x_tiles[c])
            for ob in range(G0):
                evict(ob, ps0[ob])

            # steady state: one o-block at a time, W halves prefetched
            for ob in range(G0, OB):
                whs = [load_wh(ob, h) for h in range(2)]
                ps = pp.tile([P, T_CORE], dt.float32, tag="ps", name=f"ps_{ob}")
                for c in range(DC):
                    mms(ps, whs[c // HC][:, c % HC, :], c, x_tiles[c])
                evict(ob, ps)

    nc.compile()
    return nc


def _get_nc():
    global _compiled
    if _compiled is None:
        _compiled = _build_nc()
    return _compiled


def _prep_inputs(x, W, b):
    x = np.asarray(x, dtype=np.float32)
    W = np.asarray(W, dtype=np.float32)
    b = np.asarray(b, dtype=np.float32)

    Wr = _matmul_hadU_np(W.astype(np.float64))  # [o, d] float64
    # W_pack[ob, p, c, j] = Wr[ob*128 + j, c*128 + p]
    w_pack = np.ascontiguousarray(
        Wr.reshape(OB, P, DC, P).transpose(0, 3, 2, 1).astype(np.float16)
    )
    b_pack = np.ascontiguousarray(b.reshape(OB, P).T)  # [128, 32]

    # xt[core, p, c, t] = x_core^T[c*128 + p, t]: partition-major so each
    # multi-chunk DMA reads one contiguous span per partition.
    xt = np.ascontiguousarray(
        x.reshape(N_CORES, T_CORE, D).transpose(0, 2, 1).astype(np.float16)
        .reshape(N_CORES, DC, P, T_CORE).transpose(0, 2, 1, 3)
    )

    in_maps = [
        {"xt": xt[c], "w": w_pack, "bias": b_pack} for c in range(N_CORES)
    ]
    return in_maps


def _assemble(results):
    # yt per core: [4096 o, 1024 t] -> y[t, o]
    parts = [np.ascontiguousarray(r["yt"].T) for r in results]
    y = np.concatenate(parts, axis=0)  # [8192, 4096]
    return y.reshape(4, 2048, D)


def _run(x, W, b, **spmd_kwargs):
    from concourse.bass_utils import run_bass_kernel_spmd

    nc = _get_nc()
    in_maps = _prep_inputs(x, W, b)
    res = run_bass_kernel_spmd(nc, in_maps, list(range(N_CORES)), **spmd_kwargs)
    return _assemble(res.results), res


def kernel(x, W, b):
    out, _ = _run(x, W, b)
    return out
